# revision 2
# baseline (speedup 1.0000x reference)
"""ConvTranspose2d(64->64,k4,s2,p1) + MaxPool2(2) + Hardtanh + spatial mean + tanh.

fp8 DoubleRow rewrite: each conv-transpose parity class (pp,pq) of one batch is
ONE K=256 fp8 matmul (DoubleRow: 2 k-tiles x 128 partitions = 2 dw-taps x
(2 dh-taps x 64 cin)), M=64 output channels, N=512 (8 pooled rows x 64 cols).

Host prep builds xq[128, 3, 66, 64] fp8: partition p = dh*64+c holds xpad row-
shifted by dh; the 3 col-shifted copies (i=0,1,2) give each class's two dw
windows {pq, pq+1} as a real k-tile dim (stride 66*64), so the ifmap AP is a
legal 3D [128, 2, 512].

PSUM layout: per chunk, 4 banks (one per class), partitions 0:64 = batch b0,
64:128 = batch b1. All four class maps align elementwise at the same pooled
position, so the max-combine needs NO partition crossing and NO repack DMA:
  Act : yc = copy(B10)                      (PSUM -> SBUF bf16)
  Pool: wc = clip(B11)                      (tensor_scalar min/max, per-ch bounds)
  DVE : t1 = max(B00, yc)
  Pool: t2 = max(min(B01,hi'), wc)          (scalar_tensor_tensor)
  DVE : v  = max(min(t1,hi'), t2), accum_out=acc[:,g]   (= clip(max of 4), summed)
Weights are prescaled by SW=16 to keep them in fp8e4m3 normal range; the clip
bounds/final tanh bias+scale fold the 1/SW and per-channel bias back in.
"""

import os

import numpy as np

import concourse.bass as bass
import concourse.mybir as mybir
import concourse.tile as tile

B, C, H, W = 32, 64, 64, 64
NCORES = 8
BPC = B // NCORES     # batches per core
NR = 66               # padded row count
NCHUNK = 8            # chunks per batch (8 pooled rows each)
F32 = mybir.dt.float32
F8 = mybir.dt.float8e4
PPDT = mybir.dt.bfloat16
ALU = mybir.AluOpType
DRMODE = mybir.MatmulPerfMode.DoubleRow
SW = 16.0             # weight prescale

F8NP = None  # set lazily (ml_dtypes)

PSB = int(os.environ.get("PSB", "8"))    # psum pool bufs (per tag, 2 banks each)
QB = int(os.environ.get("QB", "6"))      # tail scratch bufs
NXQ = int(os.environ.get("NXQ", "4"))    # DMA splits for x load
MODE = os.environ.get("MODE", "full")    # full | notail | noact | dma
BPHASE = int(os.environ.get("BPHASE", "3"))  # g%4 slot using the DVE-heavy tail
                                             # (4 => never, -1 => always)


def _legalize_waits(nc):
    """walrus codegen allows one sync-wait per instruction; hoist extras onto
    same-engine NoOps inserted immediately before."""
    import bass_rust
    ctr = 0
    for f in nc.m.functions:
        for blk in f.blocks:
            insts = blk.instructions
            out = []
            changed = False
            for inst in insts:
                si = inst.sync_info
                if si is not None and len(si.on_wait) > 1:
                    waits = list(si.on_wait)
                    for w in waits[:-1]:
                        nop = bass_rust.InstNoOp(
                            name=f"I-waitfix-{ctr}", ins=[], outs=[])
                        ctr += 1
                        nop.engine = inst.engine
                        nop.sync_info = mybir.SyncInfo(on_wait=[w], on_update=[])
                        out.append(nop)
                    inst.sync_info = mybir.SyncInfo(
                        on_wait=[waits[-1]], on_update=list(si.on_update))
                    changed = True
                out.append(inst)
            if changed:
                insts.clear()
                insts.extend(out)
    return ctr


def build_nc(legalize=True, loop_n=None):
    nc = bass.Bass("TRN2", target_bir_lowering=False, debug=False)
    xq_d = nc.dram_tensor("xq", [BPC, 128, 3, NR, 64], F8, kind="ExternalInput").ap()
    ws_d = nc.dram_tensor("ws", [128, 4, 2, 192], F8, kind="ExternalInput").ap()
    cs_d = nc.dram_tensor("cs", [128, 3], F32, kind="ExternalInput").ap()
    out_d = nc.dram_tensor("out", [BPC, C], F32, kind="ExternalOutput").ap()

    with tile.TileContext(nc) as tc:
        if loop_n is None:
            _body(tc, out_d, xq_d, ws_d, cs_d)
        else:
            hints = (mybir.EngineType.PE, mybir.EngineType.DVE,
                     mybir.EngineType.Activation, mybir.EngineType.Pool,
                     mybir.EngineType.SP)
            with tc.For_i(0, loop_n, 1, hint_engines=hints):
                _body(tc, out_d, xq_d, ws_d, cs_d)
    if legalize:
        _legalize_waits(nc)
    return nc


def _body(tc, out_d, xq_d, ws_d, cs_d):
    nc = tc.nc
    import contextlib
    ctx = contextlib.ExitStack()
    with ctx:
        const_pool = ctx.enter_context(tc.tile_pool(name="const", bufs=1))
        xpool = ctx.enter_context(
            tc.tile_pool(name="xq", bufs=int(os.environ.get("XB", "2"))))
        qpool = ctx.enter_context(tc.tile_pool(name="qp", bufs=QB))
        spool = ctx.enter_context(tc.tile_pool(name="sp", bufs=2))
        pspool = ctx.enter_context(tc.tile_pool(name="ps", bufs=PSB, space="PSUM"))

        w_all = const_pool.tile([128, 4, 2, 192], F8, tag="w_all")
        nc.sync.dma_start(
            w_all[:, :, :, :].rearrange("p a b c -> p (a b c)"),
            ws_d.rearrange("p a b c -> p (a b c)"))
        cs = const_pool.tile([128, 3], F32, tag="cs")
        nc.sync.dma_start(cs[:, :], cs_d)
        hi, lo, bb = cs[:, 0:1], cs[:, 1:2], cs[:, 2:3]

        xt = []
        for bi in range(BPC):
            t = xpool.tile([128, 3, NR, 64], F8, tag=f"x{bi}")
            xt.append(t)
        # few big DMAs (init-dominated otherwise): per (row-block, batch) one
        # DMA covering all 3 copies, 2KB+ contiguous runs, batches alternate
        # between the two HWDGE queues (SP + Act) so pair batches land
        # together and early.
        # few big DMAs (init-dominated otherwise): per (row-block, batch) one
        # DMA covering all 3 copies, large contiguous runs, batches alternate
        # between the two HWDGE queues (SP + Act) so pair batches land
        # together and early.
        # batch-major per queue so each pair's batches complete as early as
        # possible (b0,b1 on opposite queues land together, then b2,b3)
        rb = (NR + NXQ - 1) // NXQ
        for bi in range(BPC):
            eng = nc.sync if bi % 2 == 0 else nc.scalar
            for r0 in range(0, NR, rb):
                r1 = min(NR, r0 + rb)
                eng.dma_start(
                    xt[bi][:, :, r0:r1, :].rearrange("p i r c -> p i (r c)"),
                    xq_d[bi][:, :, r0:r1, :].rearrange("p i r c -> p i (r c)"))

        inv_n = 1.0 / (64.0 * 64.0)

        # class order: (pp,pq) indices 0..3 = (0,0),(1,0),(0,1),(1,1)
        CLASSES = [(0, 0), (1, 0), (0, 1), (1, 1)]

        # Tail identity: sum(clip(u)) = 512*hi' - sum(relu(hi'-u)) once
        # u >= lo', so the final clip+sum is one Act Relu with accum_out and
        # the whole output fold becomes out = tanh(1 - S_q*inv_n/SW).
        for p in range(BPC // 2):  # batch pairs
            b0, b1 = 2 * p, 2 * p + 1
            accq = spool.tile([128, NCHUNK], F32, tag="accq")
            for g in range(NCHUNK):
                m0 = 8 * g
                if MODE == "dma":
                    continue
                # four independent single-bank tiles so each PSUM bank frees
                # as soon as its own drain completes (fine-grained rotation
                # through all 8 banks keeps the PE fed)
                psB0 = pspool.tile([128, 512], F32, tag="ps", name="psB0")
                psB1 = pspool.tile([128, 512], F32, tag="ps", name="psB1")
                psA0 = pspool.tile([128, 512], F32, tag="ps", name="psA0")
                psA1 = pspool.tile([128, 512], F32, tag="ps", name="psA1")
                # PSUM col-tile offset 64 is illegal, so batch b1 runs as a
                # zero-padded M=128 accumulate: b0 lhsT=[w|0] (start),
                # b1 lhsT=[0|w] (stop) from the [w|0|w] width-192 layout.
                # psB first (Act's copies consume them before DVE needs psA).
                for dst, ci in ((psB0, 1), (psB1, 3), (psA0, 0), (psA1, 2)):
                    pp, pq = CLASSES[ci]
                    for half, bi in ((0, b0), (1, b1)):
                        lhsT = w_all[:, ci, :, 64 * half:64 * half + 128]
                        rhs = xt[bi][:, pq:pq + 2, m0 + pp:m0 + pp + 8, :]
                        nc.tensor.matmul(
                            dst[:, :], lhsT, rhs,
                            start=(half == 0), stop=(half == 1),
                            perf_mode=DRMODE)
                if MODE == "notail":
                    continue
                # DVE ops pay a pipeline-DRAIN penalty (~2x effective), so on
                # most chunks Act drains 3 of the 4 banks and DVE does one
                # PSUM op + cheap all-SBUF bf16 2x ops. Every BFRAC'th chunk
                # uses the Act-lighter variant to balance the engines.
                cc0 = qpool.tile([128, 512], PPDT, tag="cc0")
                cc1 = qpool.tile([128, 512], PPDT, tag="cc1")
                mm0 = qpool.tile([128, 512], PPDT, tag="mm0")
                mm1 = qpool.tile([128, 512], PPDT, tag="mm1")
                nc.scalar.copy(cc0[:, :], psB0[:, :])
                nc.scalar.copy(cc1[:, :], psB1[:, :])
                if MODE == "noact":
                    continue
                if g % 2 == 0:
                    u2 = qpool.tile([128, 2, 512], PPDT, tag="u2")
                if g % 4 == BPHASE:
                    nc.vector.scalar_tensor_tensor(
                        mm0[:, :], psA0[:, :], lo, cc0[:, :], ALU.max, ALU.max)
                    nc.vector.scalar_tensor_tensor(
                        mm1[:, :], psA1[:, :], lo, cc1[:, :], ALU.max, ALU.max)
                    nc.vector.tensor_tensor(
                        u2[:, g % 2, :], mm0[:, :], mm1[:, :], ALU.max)
                else:
                    ca0 = qpool.tile([128, 512], PPDT, tag="ca0")
                    nc.scalar.copy(ca0[:, :], psA0[:, :])
                    nc.vector.scalar_tensor_tensor(
                        mm0[:, :], psA1[:, :], lo, ca0[:, :], ALU.max, ALU.max)
                    nc.vector.scalar_tensor_tensor(
                        mm1[:, :], cc0[:, :], lo, cc1[:, :], ALU.max, ALU.max)
                    nc.vector.tensor_tensor(
                        u2[:, g % 2, :], mm0[:, :], mm1[:, :], ALU.max)
                if g % 2 == 1:
                    # q = relu(hi' - u) summed over both chunks of the pair
                    nc.scalar.activation(
                        u2[:, :, :].rearrange("p a b -> p (a b)"),
                        u2[:, :, :].rearrange("p a b -> p (a b)"),
                        mybir.ActivationFunctionType.Relu,
                        bias=hi, scale=-1.0,
                        accum_out=accq[:, g // 2:g // 2 + 1])
            S = spool.tile([128, 1], F32, tag="S")
            if MODE == "full":
                nc.vector.tensor_reduce(
                    S[:, :], accq[:, 0:NCHUNK // 2],
                    mybir.AxisListType.X, ALU.add)
            else:
                nc.vector.tensor_copy(S[:, :], cs[:, 0:1])
            T = spool.tile([128, 1], F32, tag="T")
            nc.scalar.activation(
                T[:, :], S[:, :], mybir.ActivationFunctionType.Tanh,
                bias=1.0, scale=-inv_n / SW)
            nc.sync.dma_start(out_d[2 * p:2 * p + 2, :], T[:, :])


def prep_core_inputs(x, w, b):
    """Host-side prep: pad/duplicate x (fp8), stationary-arrange w, fold b."""
    import ml_dtypes
    f8 = ml_dtypes.float8_e4m3
    x = np.asarray(x, dtype=np.float32)
    w = np.asarray(w, dtype=np.float32)
    b = np.asarray(b, dtype=np.float32)

    # weights: ws[dh*64+c, class, dw, o (+0 and +128 of a [w|0|w] 192-wide
    # band)] = SW * w[c, o, (3-pp)-2dh, (3-pq)-2dw]
    ws = np.zeros((128, 4, 2, 192), np.float32)
    classes = [(0, 0), (1, 0), (0, 1), (1, 1)]
    for ci, (pp, pq) in enumerate(classes):
        for dh in range(2):
            kh = (3 - pp) - 2 * dh
            for dw in range(2):
                kw = (3 - pq) - 2 * dw
                ws[dh * 64:dh * 64 + 64, ci, dw, 0:64] = w[:, :, kh, kw]
                ws[dh * 64:dh * 64 + 64, ci, dw, 128:192] = w[:, :, kh, kw]
    ws = (ws * SW).astype(f8)

    cs = np.zeros((128, 3), np.float32)
    bd = np.concatenate([b, b])
    cs[:, 0] = SW * (1.0 - bd)
    cs[:, 1] = SW * (-1.0 - bd)
    cs[:, 2] = bd

    # x: xpad[c, rr, ss] = x[c, rr-1, ss-1]; partition dup: group dh reads
    # xpad row r+dh at AP row r; col copies i=0,1,2: cols j+i.
    in_maps = []
    for ic in range(NCORES):
        xs = x[ic * BPC:(ic + 1) * BPC]
        xpad = np.zeros((BPC, 64, NR + 1, NR + 1), np.float32)
        xpad[:, :, 1:65, 1:65] = xs
        xq = np.zeros((BPC, 128, 3, NR, 64), np.float32)
        for dh in range(2):
            for i in range(3):
                xq[:, dh * 64:dh * 64 + 64, i, :, :] = \
                    xpad[:, :, dh:dh + NR, i:i + 64]
        in_maps.append({"xq": xq.astype(f8), "ws": ws, "cs": cs})
    return in_maps


class Runner:
    """Builds the 8-core shard_map'd executable once; callable many times."""

    def __init__(self, nc=None):
        import jax
        from jax.sharding import Mesh, PartitionSpec, NamedSharding
        try:
            from jax.experimental.shard_map import shard_map
        except ImportError:
            from jax import shard_map
        from concourse.bass2jax import (
            _bass_exec_p, partition_id_tensor, install_neuronx_cc_hook)

        install_neuronx_cc_hook()
        self.nc = nc = nc if nc is not None else build_nc()
        pname = nc.partition_id_tensor.name if nc.partition_id_tensor else None
        in_names, out_names, out_avals, zero_outs = [], [], [], []
        for alloc in nc.m.functions[0].allocations:
            if not isinstance(alloc, mybir.MemoryLocationSet):
                continue
            name = alloc.memorylocations[0].name
            if alloc.kind == "ExternalInput":
                if name != pname:
                    in_names.append(name)
            elif alloc.kind == "ExternalOutput":
                out_names.append(name)
                shape = tuple(alloc.tensor_shape)
                dtype = mybir.dt.np(alloc.dtype)
                out_avals.append(jax.core.ShapedArray(shape, dtype))
                zero_outs.append(np.zeros(shape, dtype))
        self.in_names = list(in_names)
        self.out_names = out_names
        self.zero_outs = zero_outs
        n_params, n_outs = len(in_names), len(out_names)
        all_in = in_names + out_names + ([pname] if pname else [])

        def _body_fn(*args):
            operands = list(args)
            if pname:
                operands.append(partition_id_tensor())
            return tuple(_bass_exec_p.bind(
                *operands,
                out_avals=tuple(out_avals),
                in_names=tuple(all_in),
                out_names=tuple(out_names),
                lowering_input_output_aliases=(),
                sim_require_finite=True,
                sim_require_nnan=True,
                nc=nc,
            ))

        devices = jax.devices()[:NCORES]
        self.mesh = Mesh(np.asarray(devices), ("core",))
        self.spec = PartitionSpec("core")
        self.sharding = NamedSharding(self.mesh, self.spec)
        in_specs = (self.spec,) * (n_params + n_outs)
        out_specs = (self.spec,) * n_outs
        self.fn = jax.jit(
            shard_map(_body_fn, mesh=self.mesh, in_specs=in_specs,
                      out_specs=out_specs, check_rep=False),
            donate_argnums=tuple(range(n_params, n_params + n_outs)),
            keep_unused=True,
        )
        self._jax = jax

    def stage_inputs(self, in_maps):
        concat = [np.concatenate([np.asarray(m[n]) for m in in_maps], axis=0)
                  for n in self.in_names]
        return [self._jax.device_put(a, self.sharding) for a in concat]

    def __call__(self, staged):
        zeros = [np.zeros((NCORES * z.shape[0], *z.shape[1:]), z.dtype)
                 for z in self.zero_outs]
        return self.fn(*staged, *zeros)

    def run(self, in_maps):
        outs = self(self.stage_inputs(in_maps))
        return [
            {n: np.asarray(outs[i]).reshape(NCORES, *self.zero_outs[i].shape)[c]
             for i, n in enumerate(self.out_names)}
            for c in range(NCORES)
        ]


def kernel(x: np.ndarray, w: np.ndarray, b: np.ndarray) -> np.ndarray:
    in_maps = prep_core_inputs(x, w, b)
    try:
        from concourse.bass_utils import run_bass_kernel_spmd
        nc = build_nc()
        res = run_bass_kernel_spmd(nc, in_maps, list(range(NCORES)))
        results = res.results
    except Exception:
        results = Runner().run(in_maps)
    out = np.concatenate([results[i]["out"] for i in range(NCORES)], axis=0)
    return out.reshape(B, C, 1, 1).astype(np.float32)


if __name__ == "__main__":
    rng = np.random.default_rng(0)
    x = rng.standard_normal((B, C, H, W), dtype=np.float32)
    w = rng.standard_normal((C, C, 4, 4), dtype=np.float32) * 0.05
    b = rng.standard_normal((C,), dtype=np.float32) * 0.05
    print(kernel(x, w, b).shape)
